# revision 34
# baseline (speedup 1.0000x reference)
"""Distributed TRN2 Bass kernel for AdaptiveGraphTopology pairwise edge MLP.

reference:
    a = emb @ W1a.T ; b = emb @ W1b.T           (W1a, W1b = W1[:, :H], W1[:, H:])
    hidden = relu(a[:,None,:] + b[None,:,:] + b1)      # [N,N,H]
    scores = hidden . W2[0] + b2                       # [N,N]
    weights = sigmoid(scores), zeroed diag
    mask    = (weights > 0.5) & ~eye

Sharding: rows i split across 8 cores (128 rows each); everything else
replicated. No collectives: each core DMAs out its row block, host
concatenates.

Per-core compute:
    BT[h, j] = b_j[h]        (all j)    -- f32 matmul on device
    CT[h, i] = a_i[h]+b1[h]  (local i)  -- f32 matmul + bias on device
    loop over local i:
      X_i[h, j] = relu(BT[h, j] + CT[h, i])   (DVE/ACT, fp32r out)
      scores[i, :] += w2 . X_i  via fp32r matmul whose stationary is a
      sliding window over Z[128, 256] (w2 at column 128, zeros elsewhere):
      window [128-i : 256-i] places w2 in PE column i, so row i's scores
      land in PSUM partition i and the 128 iterations accumulate a full
      [128, 1024] score block (zero columns contribute exact zeros).
    then two more accumulating matmuls (stationary -1e30*I, moving a
    per-core one-hot eye_rows matrix) push the diagonal entries to -1e30,
    so the epilogue is just: weights = sigmoid(scores+b2)  (diag -> 0.0),
    mask = scores > -b2  (diag -> 0), with no separate diagonal masking.
"""
import numpy as np

N = 1024
H = 128
NCORES = 8
ROWS = N // NCORES  # 128 rows per core

_cache = {}


def _split_multiwaits(nc, limit=1):
    """This walrus build accepts only ONE semaphore wait/update per
    instruction; Tile emits several. Split extras onto adjacent NoOps."""
    import bass_rust

    f = nc.m.functions[0]
    engines = nc.engines

    def make_nop(engine_type):
        eng = engines[engine_type]
        inst = eng.nop(nofuse=True).ins
        for b in f.blocks:
            lst = b.instructions
            for k in range(len(lst) - 1, -1, -1):
                if lst[k] is inst:
                    lst.pop(k)
                    return inst
        return inst

    n_split = 0
    for b in f.blocks:
        insts = b.instructions
        i = 0
        while i < len(insts):
            inst = insts[i]
            si = inst.sync_info
            if si is None:
                i += 1
                continue
            waits = list(si.on_wait)
            ups = list(si.on_update)
            if len(waits) <= limit and len(ups) <= 1:
                i += 1
                continue
            pre = []
            post = []
            if len(waits) > limit:
                extra, waits = waits[: len(waits) - limit], waits[len(waits) - limit :]
                for w in extra:
                    nop = make_nop(inst.engine)
                    nop.sync_info = bass_rust.SyncInfo(on_wait=[w], on_update=[])
                    pre.append(nop)
            if len(ups) > 1:
                ups, extra_u = ups[:1], ups[1:]
                for u in extra_u:
                    nop = make_nop(inst.engine)
                    nop.sync_info = bass_rust.SyncInfo(on_wait=[], on_update=[u])
                    post.append(nop)
            inst.sync_info = bass_rust.SyncInfo(on_wait=waits, on_update=ups)
            insts[i:i] = pre
            i += len(pre)
            if post:
                insts[i + 1 : i + 1] = post
            n_split += 1
            i += 1
    return n_split


def _build(reps=1, loop_reps=1, mode="full"):
    import concourse.bass as bass
    import concourse.mybir as mybir
    from concourse.tile import TileContext

    nc = bass.Bass(trn_type="TRN2")
    f32 = mybir.dt.float32
    f32r = mybir.dt.float32r
    u8 = mybir.dt.uint8

    emb_t = nc.dram_tensor("emb_t", [H, N], f32, kind="ExternalInput")
    emb_rows_t = nc.dram_tensor("emb_rows_t", [H, ROWS], f32, kind="ExternalInput")
    w1a_t = nc.dram_tensor("w1a_t", [H, H], f32, kind="ExternalInput")
    w1b_t = nc.dram_tensor("w1b_t", [H, H], f32, kind="ExternalInput")
    b1_col = nc.dram_tensor("b1_col", [H, 1], f32, kind="ExternalInput")
    zbuf = nc.dram_tensor("zbuf", [H, 2 * H], f32, kind="ExternalInput")
    b2_col = nc.dram_tensor("b2_col", [H, 1], f32, kind="ExternalInput")
    negb2_col = nc.dram_tensor("negb2_col", [H, 1], f32, kind="ExternalInput")
    # rowcol[k] = global row index of local row k: used to build the one-hot
    # eye matrix on device (iota + is_equal) that injects -BIG into the
    # diagonal score entries via one accumulating matmul
    rowcol = nc.dram_tensor("rowcol", [ROWS, 1], f32, kind="ExternalInput")
    negbig_eye = nc.dram_tensor("negbig_eye", [H, H], f32, kind="ExternalInput")

    w_out = nc.dram_tensor("w_out", [ROWS, N], f32, kind="ExternalOutput")
    m_out = nc.dram_tensor("m_out", [ROWS, N], u8, kind="ExternalOutput")

    with TileContext(nc) as tc:
        with (
            tc.tile_pool(name="const", bufs=1) as cp,
            tc.tile_pool(name="xp", bufs=14) as xp,
            tc.tile_pool(name="pp", bufs=1, space="PSUM") as pp,
        ):
            embt_s = cp.tile([H, N], f32, tag="embt")
            nc.sync.dma_start(out=embt_s[:, 0:512], in_=emb_t[:, 0:512])
            nc.sync.dma_start(out=embt_s[:, 512:1024], in_=emb_t[:, 512:1024])
            embr_s = cp.tile([H, ROWS], f32, tag="embr")
            nc.sync.dma_start(out=embr_s[:], in_=emb_rows_t[:])
            w1a_s = cp.tile([H, H], f32, tag="w1a")
            nc.sync.dma_start(out=w1a_s[:], in_=w1a_t[:])
            w1b_s = cp.tile([H, H], f32, tag="w1b")
            nc.sync.dma_start(out=w1b_s[:], in_=w1b_t[:])
            b1_s = cp.tile([H, 1], f32, tag="b1")
            nc.sync.dma_start(out=b1_s[:], in_=b1_col[:])
            z_s = cp.tile([H, 2 * H], f32, tag="z")
            nc.sync.dma_start(out=z_s[:], in_=zbuf[:])
            b2_s = cp.tile([H, 1], f32, tag="b2")
            nc.sync.dma_start(out=b2_s[:], in_=b2_col[:])
            nb2_s = cp.tile([H, 1], f32, tag="nb2")
            nc.sync.dma_start(out=nb2_s[:], in_=negb2_col[:])
            rc_s = cp.tile([ROWS, 1], f32, tag="rc")
            nc.sync.dma_start(out=rc_s[:], in_=rowcol[:])
            nbe_s = cp.tile([H, H], f32, tag="nbe")
            nc.sync.dma_start(out=nbe_s[:], in_=negbig_eye[:])

            # round f32r constants once
            zr_s = cp.tile([H, 2 * H], f32r, tag="zr")
            nc.vector.tensor_copy(zr_s[:], z_s[:])
            nber_s = cp.tile([H, H], f32r, tag="nber")
            nc.vector.tensor_copy(nber_s[:], nbe_s[:])

            # build the one-hot eye matrix on device: eyr[k, j] = (j == rowcol[k])
            it_s = cp.tile([ROWS, N], f32, tag="it")
            nc.gpsimd.iota(it_s[:], pattern=[[1, N]], base=0,
                           channel_multiplier=0,
                           allow_small_or_imprecise_dtypes=True)
            eyr_s = cp.tile([ROWS, N], f32r, tag="eyr")
            nc.vector.tensor_scalar(
                out=eyr_s[:],
                in0=it_s[:],
                scalar1=rc_s[:],
                scalar2=None,
                op0=mybir.AluOpType.is_equal,
            )

            # warm the PE HAM (clock gate) with dummy f32 matmuls while the
            # large input DMAs land, so prep + early main-loop matmuls run at
            # 2.4 GHz instead of the cold 1.2 GHz
            warm_ps = pp.tile([H, 128], f32, tag="warmp")
            for _w in range(12):
                nc.tensor.matmul(
                    warm_ps[:], w1a_s[:], w1a_s[:], start=True, stop=True
                )

            # force the sigmoid ACT table set to load during prep, so the
            # epilogue sigmoid doesn't pay a ~2.7us mid-kernel table swap
            # (relu/identity are filler entries in every set); reading
            # warm_ps also keeps the warm matmuls alive through DCE
            warm_s = cp.tile([H, 1], f32, tag="warm")
            nc.scalar.activation(
                warm_s[:], warm_ps[:, 0:1], mybir.ActivationFunctionType.Sigmoid
            )

            if loop_reps > 1:
                with tc.For_i(0, loop_reps, 1):
                    _body_once(nc, tc, cp, xp, pp, mybir, f32, f32r, u8,
                               embt_s, embr_s, w1a_s, w1b_s, b1_s, zr_s, b2_s,
                               nb2_s, eyr_s, nber_s, w_out, m_out, mode)
            else:
                for _rep in range(reps):
                    _body_once(nc, tc, cp, xp, pp, mybir, f32, f32r, u8,
                               embt_s, embr_s, w1a_s, w1b_s, b1_s, zr_s, b2_s,
                               nb2_s, eyr_s, nber_s, w_out, m_out, mode)

    _split_multiwaits(nc)
    return nc


def _body_once(nc, tc, cp, xp, pp, mybir, f32, f32r, u8,
               embt_s, embr_s, w1a_s, w1b_s, b1_s, zr_s, b2_s, nb2_s,
               eyr_s, nber_s, w_out, m_out, mode="full"):
    if mode == "empty":
        return
    if True:
        if True:
            # BT = W1b @ embT  (f32, exact): psum half per matmul
            bt_ps = pp.tile([H, N], f32, tag="btp")
            nc.tensor.matmul(
                bt_ps[:, 0:512], w1b_s[:], embt_s[:, 0:512], start=True, stop=True
            )
            nc.tensor.matmul(
                bt_ps[:, 512:1024], w1b_s[:], embt_s[:, 512:1024], start=True, stop=True
            )
            bt_s = cp.tile([H, N], f32, tag="bt")
            nc.vector.tensor_copy(bt_s[:, 0:512], bt_ps[:, 0:512])
            nc.vector.tensor_copy(bt_s[:, 512:1024], bt_ps[:, 512:1024])

            # CT = W1a @ embT_rows + b1  (f32, exact)
            ct_ps = pp.tile([H, ROWS], f32, tag="ctp")
            nc.tensor.matmul(ct_ps[:], w1a_s[:], embr_s[:], start=True, stop=True)
            ct_s = cp.tile([H, ROWS], f32, tag="ct")
            nc.scalar.activation(
                ct_s[:], ct_ps[:], mybir.ActivationFunctionType.Identity, bias=b1_s[:]
            )

            # main loop: accumulate scores into PSUM [128 rows, 1024 cols]
            sc_ps = pp.tile([ROWS, N], f32, tag="scores")
            # initialize each scores bank with -BIG at the diagonal entries
            # (zeros elsewhere): out[k, j] = -BIG*eye[k, j]; keeps the
            # epilogue off the critical tail
            for h0 in (0, 512):
                nc.tensor.matmul(
                    sc_ps[:, h0 : h0 + 512],
                    nber_s[:],
                    eyr_s[:, h0 : h0 + 512],
                    start=True,
                    stop=False,
                )
            xfix = None
            if mode == "nogen":
                xfix = cp.tile([H, N], f32r, tag="xfix")
                nc.vector.tensor_copy(xfix[:, 0:256], zr_s[:])
            if mode == "full2":
                # col-group tiled reduction: 32-wide stationaries, 4 strips
                for k in range(32):
                    for g in range(4):
                        i = 32 * g + k
                        x = xp.tile([H, N], f32r, tag="x")
                        if (i * 5) % 13 < 5:
                            nc.scalar.activation(
                                x[:],
                                bt_s[:],
                                mybir.ActivationFunctionType.Relu,
                                bias=ct_s[:, i : i + 1],
                            )
                        else:
                            nc.vector.tensor_scalar(
                                out=x[:],
                                in0=bt_s[:],
                                scalar1=ct_s[:, i : i + 1],
                                scalar2=0.0,
                                op0=mybir.AluOpType.add,
                                op1=mybir.AluOpType.max,
                            )
                        lhsT = zr_s[:, H - k : H + 32 - k]
                        for h0 in (0, 512):
                            nc.tensor.matmul(
                                sc_ps[32 * g : 32 * g + 32, h0 : h0 + 512],
                                lhsT,
                                x[:, h0 : h0 + 512],
                                start=(k == 0),
                                stop=(k == 31),
                                tile_position=(0, 32 * g),
                            )
                _epilogue(nc, cp, mybir, f32, u8, sc_ps, b2_s, nb2_s, w_out, m_out)
                return

            for i in range(ROWS):
                if mode != "nogen":
                    x = xp.tile([H, N], f32r, tag="x")
                    if mode == "actgen" or (mode != "dvegen" and i % 3 == 1):
                        # ACT path: relu(in + bias), ~1147ns
                        nc.scalar.activation(
                            x[:],
                            bt_s[:],
                            mybir.ActivationFunctionType.Relu,
                            bias=ct_s[:, i : i + 1],
                        )
                    else:
                        # DVE path: (in + c_i) then max(.,0), ~720ns
                        nc.vector.tensor_scalar(
                            out=x[:],
                            in0=bt_s[:],
                            scalar1=ct_s[:, i : i + 1],
                            scalar2=0.0,
                            op0=mybir.AluOpType.add,
                            op1=mybir.AluOpType.max,
                        )
                else:
                    x = xfix
                if mode == "nomm":
                    continue
                lhsT = zr_s[:, H - i : 2 * H - i]
                nc.tensor.matmul(
                    sc_ps[:, 0:512],
                    lhsT,
                    x[:, 0:512],
                    start=False,
                    stop=(i == ROWS - 1),
                )
                nc.tensor.matmul(
                    sc_ps[:, 512:1024],
                    lhsT,
                    x[:, 512:1024],
                    start=False,
                    stop=(i == ROWS - 1),
                )
            if mode == "nomm":
                return

            _epilogue(nc, cp, mybir, f32, u8, sc_ps, b2_s, nb2_s, w_out, m_out)


def _epilogue(nc, cp, mybir, f32, u8, sc_ps, b2_s, nb2_s, w_out, m_out):
    # diagonal score entries hold -BIG: sigmoid -> 0 weight, is_gt -> 0 mask
    sig_s = cp.tile([ROWS, N], f32, tag="sig")
    nc.scalar.activation(
        sig_s[:], sc_ps[:], mybir.ActivationFunctionType.Sigmoid, bias=b2_s[:]
    )
    nc.sync.dma_start(out=w_out[:], in_=sig_s[:])

    m_s = cp.tile([ROWS, N], u8, tag="m")
    nc.vector.tensor_scalar(
        out=m_s[:],
        in0=sc_ps[:],
        scalar1=nb2_s[:],
        scalar2=None,
        op0=mybir.AluOpType.is_gt,
    )
    nc.sync.dma_start(out=m_out[:], in_=m_s[:])


def _build_in_maps(inputs):
    node_emb = np.asarray(inputs["node_emb"], dtype=np.float32)
    W1 = np.asarray(inputs["W1"], dtype=np.float32)
    b1 = np.asarray(inputs["b1"], dtype=np.float32)
    W2 = np.asarray(inputs["W2"], dtype=np.float32)
    b2 = np.asarray(inputs["b2"], dtype=np.float32)

    emb_t = np.ascontiguousarray(node_emb.T)  # [H, N]
    w1a_t = np.ascontiguousarray(W1[:, :H].T)  # [e, h]
    w1b_t = np.ascontiguousarray(W1[:, H:].T)
    b1_col = np.ascontiguousarray(b1.reshape(H, 1))
    zbuf = np.zeros((H, 2 * H), dtype=np.float32)
    zbuf[:, H] = W2[0]
    b2v = np.float32(b2.reshape(-1)[0])
    b2_col = np.full((H, 1), b2v, dtype=np.float32)
    negb2_col = -b2_col

    negbig_eye = np.zeros((H, H), dtype=np.float32)
    np.fill_diagonal(negbig_eye, np.float32(-1e30))

    in_maps = []
    for c in range(NCORES):
        r0 = c * ROWS
        in_maps.append(
            {
                "emb_t": emb_t,
                "emb_rows_t": np.ascontiguousarray(emb_t[:, r0 : r0 + ROWS]),
                "w1a_t": w1a_t,
                "w1b_t": w1b_t,
                "b1_col": b1_col,
                "zbuf": zbuf,
                "b2_col": b2_col,
                "negb2_col": negb2_col,
                "rowcol": (r0 + np.arange(ROWS, dtype=np.float32)).reshape(ROWS, 1),
                "negbig_eye": negbig_eye,
            }
        )
    return in_maps


def kernel(node_emb, W1, b1, W2, b2, temperature=None, **_ignored):
    import time

    from concourse.bass_utils import run_bass_kernel_spmd

    if "nc" not in _cache:
        _cache["nc"] = _build()
    nc = _cache["nc"]

    in_maps = _build_in_maps(
        {"node_emb": node_emb, "W1": W1, "b1": b1, "W2": W2, "b2": b2}
    )
    # the device occasionally reports NRT_EXEC_UNIT_UNRECOVERABLE if a prior
    # process wedged it; it self-recovers after ~30s, so retry those (and only
    # those) with backoff
    for attempt in range(3):
        try:
            res = run_bass_kernel_spmd(nc, in_maps, core_ids=list(range(NCORES)))
            break
        except Exception as e:  # noqa: BLE001
            msg = str(e)
            transient = (
                "UNRECOVERABLE" in msg
                or "unrecoverable" in msg
                or "UNAVAILABLE" in msg
            )
            if attempt == 2 or not transient:
                raise
            time.sleep(30 * (attempt + 1))
    weights = np.concatenate([res.results[c]["w_out"] for c in range(NCORES)], axis=0)
    mask = np.concatenate([res.results[c]["m_out"] for c in range(NCORES)], axis=0).astype(bool)
    return weights, mask


# revision 35
# speedup vs baseline: 1.1038x; 1.1038x over previous
"""Distributed TRN2 Bass kernel for AdaptiveGraphTopology pairwise edge MLP.

reference:
    a = emb @ W1a.T ; b = emb @ W1b.T           (W1a, W1b = W1[:, :H], W1[:, H:])
    hidden = relu(a[:,None,:] + b[None,:,:] + b1)      # [N,N,H]
    scores = hidden . W2[0] + b2                       # [N,N]
    weights = sigmoid(scores), zeroed diag
    mask    = (weights > 0.5) & ~eye

Sharding: rows i split across 8 cores (128 rows each); everything else
replicated. No collectives: each core DMAs out its row block, host
concatenates.

Per-core compute:
    BT[h, j] = b_j[h]        (all j)    -- f32 matmul on device
    CT[h, i] = a_i[h]+b1[h]  (local i)  -- f32 matmul + bias on device
    loop over local i:
      X_i[h, j] = relu(BT[h, j] + CT[h, i])   (DVE/ACT, fp32r out)
      scores[i, :] += w2 . X_i  via fp32r matmul whose stationary is a
      sliding window over Z[128, 256] (w2 at column 128, zeros elsewhere):
      window [128-i : 256-i] places w2 in PE column i, so row i's scores
      land in PSUM partition i and the 128 iterations accumulate a full
      [128, 1024] score block (zero columns contribute exact zeros).
    then two more accumulating matmuls (stationary -1e30*I, moving a
    per-core one-hot eye_rows matrix) push the diagonal entries to -1e30,
    so the epilogue is just: weights = sigmoid(scores+b2)  (diag -> 0.0),
    mask = scores > -b2  (diag -> 0), with no separate diagonal masking.
"""
import numpy as np

N = 1024
H = 128
NCORES = 8
ROWS = N // NCORES  # 128 rows per core

_cache = {}


def _split_multiwaits(nc, limit=1):
    """This walrus build accepts only ONE semaphore wait/update per
    instruction; Tile emits several. Split extras onto adjacent NoOps."""
    import bass_rust

    f = nc.m.functions[0]
    engines = nc.engines

    def make_nop(engine_type):
        eng = engines[engine_type]
        inst = eng.nop(nofuse=True).ins
        for b in f.blocks:
            lst = b.instructions
            for k in range(len(lst) - 1, -1, -1):
                if lst[k] is inst:
                    lst.pop(k)
                    return inst
        return inst

    n_split = 0
    for b in f.blocks:
        insts = b.instructions
        i = 0
        while i < len(insts):
            inst = insts[i]
            si = inst.sync_info
            if si is None:
                i += 1
                continue
            waits = list(si.on_wait)
            ups = list(si.on_update)
            if len(waits) <= limit and len(ups) <= 1:
                i += 1
                continue
            pre = []
            post = []
            if len(waits) > limit:
                extra, waits = waits[: len(waits) - limit], waits[len(waits) - limit :]
                for w in extra:
                    nop = make_nop(inst.engine)
                    nop.sync_info = bass_rust.SyncInfo(on_wait=[w], on_update=[])
                    pre.append(nop)
            if len(ups) > 1:
                ups, extra_u = ups[:1], ups[1:]
                for u in extra_u:
                    nop = make_nop(inst.engine)
                    nop.sync_info = bass_rust.SyncInfo(on_wait=[], on_update=[u])
                    post.append(nop)
            inst.sync_info = bass_rust.SyncInfo(on_wait=waits, on_update=ups)
            insts[i:i] = pre
            i += len(pre)
            if post:
                insts[i + 1 : i + 1] = post
            n_split += 1
            i += 1
    return n_split


def _build(reps=1, loop_reps=1, mode="full"):
    import concourse.bass as bass
    import concourse.mybir as mybir
    from concourse.tile import TileContext

    nc = bass.Bass(trn_type="TRN2")
    f32 = mybir.dt.float32
    f32r = mybir.dt.float32r
    u8 = mybir.dt.uint8

    emb_t = nc.dram_tensor("emb_t", [H, N], f32, kind="ExternalInput")
    emb_rows_t = nc.dram_tensor("emb_rows_t", [H, ROWS], f32, kind="ExternalInput")
    w1a_t = nc.dram_tensor("w1a_t", [H, H], f32, kind="ExternalInput")
    w1b_t = nc.dram_tensor("w1b_t", [H, H], f32, kind="ExternalInput")
    b1_col = nc.dram_tensor("b1_col", [H, 1], f32, kind="ExternalInput")
    zbuf = nc.dram_tensor("zbuf", [H, 2 * H], f32, kind="ExternalInput")
    b2_col = nc.dram_tensor("b2_col", [H, 1], f32, kind="ExternalInput")
    negb2_col = nc.dram_tensor("negb2_col", [H, 1], f32, kind="ExternalInput")
    # rowcol[k] = global row index of local row k: used to build the one-hot
    # eye matrix on device (iota + is_equal) that injects -BIG into the
    # diagonal score entries via one accumulating matmul
    rowcol = nc.dram_tensor("rowcol", [ROWS, 1], f32, kind="ExternalInput")
    negbig_eye = nc.dram_tensor("negbig_eye", [H, H], f32, kind="ExternalInput")

    w_out = nc.dram_tensor("w_out", [ROWS, N], f32, kind="ExternalOutput")
    m_out = nc.dram_tensor("m_out", [ROWS, N], u8, kind="ExternalOutput")

    with TileContext(nc) as tc:
        with (
            tc.tile_pool(name="const", bufs=1) as cp,
            tc.tile_pool(name="xp", bufs=14) as xp,
            tc.tile_pool(name="pp", bufs=1, space="PSUM") as pp,
        ):
            emba_s = cp.tile([H, 512], f32, tag="emba")
            nc.sync.dma_start(out=emba_s[:], in_=emb_t[:, 0:512])
            embb_s = cp.tile([H, 512], f32, tag="embb")
            nc.sync.dma_start(out=embb_s[:], in_=emb_t[:, 512:1024])
            embr_s = cp.tile([H, ROWS], f32, tag="embr")
            nc.sync.dma_start(out=embr_s[:], in_=emb_rows_t[:])
            w1a_s = cp.tile([H, H], f32, tag="w1a")
            nc.sync.dma_start(out=w1a_s[:], in_=w1a_t[:])
            w1b_s = cp.tile([H, H], f32, tag="w1b")
            nc.sync.dma_start(out=w1b_s[:], in_=w1b_t[:])
            b1_s = cp.tile([H, 1], f32, tag="b1")
            nc.sync.dma_start(out=b1_s[:], in_=b1_col[:])
            z_s = cp.tile([H, 2 * H], f32, tag="z")
            nc.sync.dma_start(out=z_s[:], in_=zbuf[:])
            b2_s = cp.tile([H, 1], f32, tag="b2")
            nc.sync.dma_start(out=b2_s[:], in_=b2_col[:])
            nb2_s = cp.tile([H, 1], f32, tag="nb2")
            nc.sync.dma_start(out=nb2_s[:], in_=negb2_col[:])
            rc_s = cp.tile([ROWS, 1], f32, tag="rc")
            nc.sync.dma_start(out=rc_s[:], in_=rowcol[:])
            nbe_s = cp.tile([H, H], f32, tag="nbe")
            nc.sync.dma_start(out=nbe_s[:], in_=negbig_eye[:])

            # round f32r constants once
            zr_s = cp.tile([H, 2 * H], f32r, tag="zr")
            nc.vector.tensor_copy(zr_s[:], z_s[:])
            nber_s = cp.tile([H, H], f32r, tag="nber")
            nc.vector.tensor_copy(nber_s[:], nbe_s[:])

            # build the one-hot eye matrix on device: eyr[k, j] = (j == rowcol[k])
            it_s = cp.tile([ROWS, N], f32, tag="it")
            nc.gpsimd.iota(it_s[:], pattern=[[1, N]], base=0,
                           channel_multiplier=0,
                           allow_small_or_imprecise_dtypes=True)
            eyr_s = cp.tile([ROWS, N], f32r, tag="eyr")
            nc.vector.tensor_scalar(
                out=eyr_s[:],
                in0=it_s[:],
                scalar1=rc_s[:],
                scalar2=None,
                op0=mybir.AluOpType.is_equal,
            )

            # warm the PE HAM (clock gate) with dummy f32 matmuls while the
            # large input DMAs land, so prep + early main-loop matmuls run at
            # 2.4 GHz instead of the cold 1.2 GHz
            warm_ps = pp.tile([H, 128], f32, tag="warmp")
            for _w in range(12):
                nc.tensor.matmul(
                    warm_ps[:], w1a_s[:], w1a_s[:], start=True, stop=True
                )

            # force the sigmoid ACT table set to load during prep, so the
            # epilogue sigmoid doesn't pay a ~2.7us mid-kernel table swap
            # (relu/identity are filler entries in every set); reading
            # warm_ps also keeps the warm matmuls alive through DCE
            warm_s = cp.tile([H, 1], f32, tag="warm")
            nc.scalar.activation(
                warm_s[:], warm_ps[:, 0:1], mybir.ActivationFunctionType.Sigmoid
            )

            if loop_reps > 1:
                with tc.For_i(0, loop_reps, 1):
                    _body_once(nc, tc, cp, xp, pp, mybir, f32, f32r, u8,
                               (emba_s, embb_s), embr_s, w1a_s, w1b_s, b1_s,
                               zr_s, b2_s, nb2_s, eyr_s, nber_s, w_out, m_out,
                               mode)
            else:
                for _rep in range(reps):
                    _body_once(nc, tc, cp, xp, pp, mybir, f32, f32r, u8,
                               (emba_s, embb_s), embr_s, w1a_s, w1b_s, b1_s,
                               zr_s, b2_s, nb2_s, eyr_s, nber_s, w_out, m_out,
                               mode)

    _split_multiwaits(nc)
    return nc


def _body_once(nc, tc, cp, xp, pp, mybir, f32, f32r, u8,
               embt_halves, embr_s, w1a_s, w1b_s, b1_s, zr_s, b2_s, nb2_s,
               eyr_s, nber_s, w_out, m_out, mode="full"):
    emba_s, embb_s = embt_halves
    if mode == "empty":
        return
    if True:
        if True:
            # BT = W1b @ embT  (f32, exact): psum half per matmul; each half
            # depends only on its own emb DMA, and the PSUM->SBUF copies run
            # on different engines so they overlap
            bt_ps = pp.tile([H, N], f32, tag="btp")
            nc.tensor.matmul(
                bt_ps[:, 0:512], w1b_s[:], emba_s[:], start=True, stop=True
            )
            nc.tensor.matmul(
                bt_ps[:, 512:1024], w1b_s[:], embb_s[:], start=True, stop=True
            )
            bt_s = cp.tile([H, N], f32, tag="bt")
            nc.vector.tensor_copy(bt_s[:, 0:512], bt_ps[:, 0:512])
            nc.scalar.copy(bt_s[:, 512:1024], bt_ps[:, 512:1024])

            # CT = W1a @ embT_rows + b1  (f32, exact)
            ct_ps = pp.tile([H, ROWS], f32, tag="ctp")
            nc.tensor.matmul(ct_ps[:], w1a_s[:], embr_s[:], start=True, stop=True)
            ct_s = cp.tile([H, ROWS], f32, tag="ct")
            nc.scalar.activation(
                ct_s[:], ct_ps[:], mybir.ActivationFunctionType.Identity, bias=b1_s[:]
            )

            # main loop: accumulate scores into PSUM [128 rows, 1024 cols]
            sc_ps = pp.tile([ROWS, N], f32, tag="scores")
            # initialize each scores bank with -BIG at the diagonal entries
            # (zeros elsewhere): out[k, j] = -BIG*eye[k, j]; keeps the
            # epilogue off the critical tail
            for h0 in (0, 512):
                nc.tensor.matmul(
                    sc_ps[:, h0 : h0 + 512],
                    nber_s[:],
                    eyr_s[:, h0 : h0 + 512],
                    start=True,
                    stop=False,
                )
            xfix = None
            if mode == "nogen":
                xfix = cp.tile([H, N], f32r, tag="xfix")
                nc.vector.tensor_copy(xfix[:, 0:256], zr_s[:])
            if mode == "full2":
                # col-group tiled reduction: 32-wide stationaries, 4 strips
                for k in range(32):
                    for g in range(4):
                        i = 32 * g + k
                        x = xp.tile([H, N], f32r, tag="x")
                        if (i * 5) % 13 < 5:
                            nc.scalar.activation(
                                x[:],
                                bt_s[:],
                                mybir.ActivationFunctionType.Relu,
                                bias=ct_s[:, i : i + 1],
                            )
                        else:
                            nc.vector.tensor_scalar(
                                out=x[:],
                                in0=bt_s[:],
                                scalar1=ct_s[:, i : i + 1],
                                scalar2=0.0,
                                op0=mybir.AluOpType.add,
                                op1=mybir.AluOpType.max,
                            )
                        lhsT = zr_s[:, H - k : H + 32 - k]
                        for h0 in (0, 512):
                            nc.tensor.matmul(
                                sc_ps[32 * g : 32 * g + 32, h0 : h0 + 512],
                                lhsT,
                                x[:, h0 : h0 + 512],
                                start=(k == 0),
                                stop=(k == 31),
                                tile_position=(0, 32 * g),
                            )
                _epilogue(nc, cp, mybir, f32, u8, sc_ps, b2_s, nb2_s, w_out, m_out)
                return

            for i in range(ROWS):
                if mode != "nogen":
                    x = xp.tile([H, N], f32r, tag="x")
                    if mode == "actgen" or (mode != "dvegen" and i % 3 == 1):
                        # ACT path: relu(in + bias), ~1147ns
                        nc.scalar.activation(
                            x[:],
                            bt_s[:],
                            mybir.ActivationFunctionType.Relu,
                            bias=ct_s[:, i : i + 1],
                        )
                    else:
                        # DVE path: (in + c_i) then max(.,0), ~720ns
                        nc.vector.tensor_scalar(
                            out=x[:],
                            in0=bt_s[:],
                            scalar1=ct_s[:, i : i + 1],
                            scalar2=0.0,
                            op0=mybir.AluOpType.add,
                            op1=mybir.AluOpType.max,
                        )
                else:
                    x = xfix
                if mode == "nomm":
                    continue
                lhsT = zr_s[:, H - i : 2 * H - i]
                nc.tensor.matmul(
                    sc_ps[:, 0:512],
                    lhsT,
                    x[:, 0:512],
                    start=False,
                    stop=(i == ROWS - 1),
                )
                nc.tensor.matmul(
                    sc_ps[:, 512:1024],
                    lhsT,
                    x[:, 512:1024],
                    start=False,
                    stop=(i == ROWS - 1),
                )
            if mode == "nomm":
                return

            _epilogue(nc, cp, mybir, f32, u8, sc_ps, b2_s, nb2_s, w_out, m_out)


def _epilogue(nc, cp, mybir, f32, u8, sc_ps, b2_s, nb2_s, w_out, m_out):
    # diagonal score entries hold -BIG: sigmoid -> 0 weight, is_gt -> 0 mask
    sig_s = cp.tile([ROWS, N], f32, tag="sig")
    nc.scalar.activation(
        sig_s[:], sc_ps[:], mybir.ActivationFunctionType.Sigmoid, bias=b2_s[:]
    )
    nc.sync.dma_start(out=w_out[:], in_=sig_s[:])

    m_s = cp.tile([ROWS, N], u8, tag="m")
    nc.vector.tensor_scalar(
        out=m_s[:],
        in0=sc_ps[:],
        scalar1=nb2_s[:],
        scalar2=None,
        op0=mybir.AluOpType.is_gt,
    )
    nc.sync.dma_start(out=m_out[:], in_=m_s[:])


def _build_in_maps(inputs):
    node_emb = np.asarray(inputs["node_emb"], dtype=np.float32)
    W1 = np.asarray(inputs["W1"], dtype=np.float32)
    b1 = np.asarray(inputs["b1"], dtype=np.float32)
    W2 = np.asarray(inputs["W2"], dtype=np.float32)
    b2 = np.asarray(inputs["b2"], dtype=np.float32)

    emb_t = np.ascontiguousarray(node_emb.T)  # [H, N]
    w1a_t = np.ascontiguousarray(W1[:, :H].T)  # [e, h]
    w1b_t = np.ascontiguousarray(W1[:, H:].T)
    b1_col = np.ascontiguousarray(b1.reshape(H, 1))
    zbuf = np.zeros((H, 2 * H), dtype=np.float32)
    zbuf[:, H] = W2[0]
    b2v = np.float32(b2.reshape(-1)[0])
    b2_col = np.full((H, 1), b2v, dtype=np.float32)
    negb2_col = -b2_col

    negbig_eye = np.zeros((H, H), dtype=np.float32)
    np.fill_diagonal(negbig_eye, np.float32(-1e30))

    in_maps = []
    for c in range(NCORES):
        r0 = c * ROWS
        in_maps.append(
            {
                "emb_t": emb_t,
                "emb_rows_t": np.ascontiguousarray(emb_t[:, r0 : r0 + ROWS]),
                "w1a_t": w1a_t,
                "w1b_t": w1b_t,
                "b1_col": b1_col,
                "zbuf": zbuf,
                "b2_col": b2_col,
                "negb2_col": negb2_col,
                "rowcol": (r0 + np.arange(ROWS, dtype=np.float32)).reshape(ROWS, 1),
                "negbig_eye": negbig_eye,
            }
        )
    return in_maps


def kernel(node_emb, W1, b1, W2, b2, temperature=None, **_ignored):
    import time

    from concourse.bass_utils import run_bass_kernel_spmd

    if "nc" not in _cache:
        _cache["nc"] = _build()
    nc = _cache["nc"]

    in_maps = _build_in_maps(
        {"node_emb": node_emb, "W1": W1, "b1": b1, "W2": W2, "b2": b2}
    )
    # the device occasionally reports NRT_EXEC_UNIT_UNRECOVERABLE if a prior
    # process wedged it; it self-recovers after ~30s, so retry those (and only
    # those) with backoff
    for attempt in range(3):
        try:
            res = run_bass_kernel_spmd(nc, in_maps, core_ids=list(range(NCORES)))
            break
        except Exception as e:  # noqa: BLE001
            msg = str(e)
            transient = (
                "UNRECOVERABLE" in msg
                or "unrecoverable" in msg
                or "UNAVAILABLE" in msg
            )
            if attempt == 2 or not transient:
                raise
            time.sleep(30 * (attempt + 1))
    weights = np.concatenate([res.results[c]["w_out"] for c in range(NCORES)], axis=0)
    mask = np.concatenate([res.results[c]["m_out"] for c in range(NCORES)], axis=0).astype(bool)
    return weights, mask
